# revision 29
# baseline (speedup 1.0000x reference)
"""Trainium2 Bass kernel for nn_MeshConv (COO SpMM + 128x128 Linear).

out[r, :] = (sum_{e: rows[e]==r} vals[e] * x[cols[e], :]) @ W.T + b

Strategy (8 NeuronCores, one SPMD program):
  - Row-shard: core c owns output rows [c*12500, (c+1)*12500); x, W, b
    are replicated per core, so no collectives are needed.
  - Host packs each core's edges by (128-row output window, column
    chunk) into 128-edge tiles.  Chunk boundaries are tuned so the four
    per-batch gather calls are balanced and tile padding is minimal
    (each chunk <= 32768 rows for int16 gather indices).
  - ALL metadata planes (wrapped gather indices, vals, local rows) are
    SBUF-resident up front, so per-batch work has no metadata DMA
    dependency and gather drains run back-to-back.  Each chunk's gather
    is split into 2 sub-calls for finer SWDGE ring pipelining.
  - Per batch: dma_gather x rows (bf16, SWDGE, 4 queues) into SBUF;
    build the selection matrix S[e, r] = (iota_r == lrow_e) * val_e
    split across DVE (is_equal+mult on KS-column groups) and ScalarE
    (Square/Relu/mul per column, ACT_FRAC of columns); accumulate
    aggT[cin, rows] = X_tile^T @ S_tile per window in PSUM on TensorE.
  - Projection out_w = aggT.T @ W.T + bias: the bias lands via a rank-1
    matmul into the same PSUM bank and ScalarE does PSUM->SBUF copies,
    keeping DVE dedicated to S builds.
"""

import os
import sys

for _p in ("/opt/trn_rl_repo",):
    if _p not in sys.path:
        sys.path.insert(0, _p)

import numpy as np

# --- problem constants (from the problem spec) ---
N_NODES = 100000
C = 128
N_CORES = 8
RPC = N_NODES // N_CORES          # rows per core: 12500
WIN = 128                         # output window = PSUM partition dim
# column-chunk boundaries (each segment <= 32768 for int16 gather indices);
# sizes tuned to minimize 128-edge tile padding for this edge distribution
CHUNK_BOUNDS = [0, 28120, 55880, 72060, 100000]
CB = int(os.environ.get("MESHCONV_CB", "88"))

EDGE_DTYPE = os.environ.get("MESHCONV_EDGE_DTYPE", "bf16")
KS = 16                           # S-build tiles per DVE op
NSWQ = int(os.environ.get("MESHCONV_NSWQ", "4"))
XG_BUFS = int(os.environ.get("MESHCONV_XG_BUFS", "3"))
SM_BUFS = int(os.environ.get("MESHCONV_SM_BUFS", "3"))
PSUM_BUFS = int(os.environ.get("MESHCONV_PSUM_BUFS", "4"))
SUBCALLS = int(os.environ.get("MESHCONV_SUBCALLS", "2"))  # gather sub-calls per chunk
ACT_FRAC = float(os.environ.get("MESHCONV_ACT_FRAC", "0.23"))  # S-build share on ScalarE
TSCOL = os.environ.get("MESHCONV_TSCOL", "0") == "1"  # per-column tensor_scalar S build
TRS = os.environ.get("MESHCONV_TRS", "0") == "1"  # transposed S-group build (DVE 2x)

TRACE = False          # set by test.py for profiling runs
LAST_RESULT = {}       # test.py reads exec_time_ns etc. from here


def _derived():
    nw = (RPC + WIN - 1) // WIN
    nk = len(CHUNK_BOUNDS) - 1
    return nw, nk


def _host_prep(rows, cols, vals):
    """Pack edges per (core, window, chunk) into fixed 128-lane tiles."""
    NW, NK = _derived()
    rows = np.asarray(rows).astype(np.int64)
    cols = np.asarray(cols).astype(np.int64)
    vals = np.asarray(vals).astype(np.float32)

    core = rows // RPC
    lrow_full = rows - core * RPC
    win = lrow_full // WIN
    lrow = lrow_full - win * WIN
    bounds = np.asarray(CHUNK_BOUNDS)
    chunk = np.searchsorted(bounds, cols, side="right") - 1
    cidx = cols - bounds[chunk]

    gid = (core * NW + win) * NK + chunk
    cnt = np.bincount(gid, minlength=N_CORES * NW * NK).reshape(N_CORES, NW, NK)
    t_wk = -(-cnt.max(axis=0) // 128)         # [NW, NK]
    tw_tot = t_wk.sum(axis=1)
    for w in np.flatnonzero(tw_tot == 0):
        t_wk[w, 0] = 1
    tw_tot = t_wk.sum(axis=1)

    batches = []  # (w0, nwin, ncols)
    w = 0
    while w < NW:
        w0, ccols = w, 0
        while w < NW and ccols + tw_tot[w] <= CB:
            ccols += int(tw_tot[w])
            w += 1
        assert w > w0, f"window {w0} needs {tw_tot[w0]} > CB={CB} columns"
        batches.append((w0, w - w0, ccols))

    col_of = np.zeros((NW, NK), dtype=np.int64)
    calls = []  # (batch_idx, k, col_base, ncols)
    base = 0
    for bi, (w0, nwin, _) in enumerate(batches):
        for k in range(NK):
            cb = base
            for w in range(w0, w0 + nwin):
                col_of[w, k] = base
                base += int(t_wk[w, k])
            if base > cb:
                calls.append((bi, k, cb, base - cb))
    tc_total = int(base)

    order = np.lexsort((chunk, win, core))
    core_s, win_s, chunk_s = core[order], win[order], chunk[order]
    grp = (core_s * NW + win_s) * NK + chunk_s
    start_of_grp = np.searchsorted(grp, np.arange(N_CORES * NW * NK), side="left")
    rank = np.arange(len(grp)) - start_of_grp[grp]
    t = rank // 128
    p = rank - t * 128
    gcol = col_of[win_s, chunk_s] + t

    sidx = np.zeros((N_CORES, tc_total, 128), dtype=np.int16)
    sval = np.zeros((N_CORES, tc_total, 128), dtype=np.float32)
    slrow = np.zeros((N_CORES, tc_total, 128), dtype=np.float32)
    sidx[core_s, gcol, p] = cidx[order].astype(np.int16)
    sval[core_s, gcol, p] = vals[order]
    # stored negated: DVE compares against a negated iota plane, and the
    # ScalarE path uses it directly as the bias in Square(iota - lrow)
    slrow[core_s, gcol, p] = -lrow[order].astype(np.float32)

    # wrapped int16 index plane, per gather-call region (sub-call aware)
    eidx16 = np.zeros((N_CORES, 128, tc_total * 8), dtype=np.int16)
    sub_regions = []
    for _, _, cb, ck in calls:
        if SUBCALLS <= 1:
            sub_regions.append((cb, ck))
        else:
            step = -(-ck // SUBCALLS)
            o = cb
            while o < cb + ck:
                sub_regions.append((o, min(step, cb + ck - o)))
                o += step
    for cb, ck in sub_regions:
        flat = sidx[:, cb : cb + ck, :].reshape(N_CORES, ck * 128)
        wrapped = flat.reshape(N_CORES, ck * 8, 16).transpose(0, 2, 1)
        eidx16[:, :, cb * 8 : (cb + ck) * 8] = np.tile(wrapped, (1, 8, 1))

    ev = np.ascontiguousarray(sval.transpose(0, 2, 1))    # [NC, 128, TC]
    el = np.ascontiguousarray(slrow.transpose(0, 2, 1))

    win_cols = [
        [int(col_of[w, k]) + t for k in range(NK) for t in range(int(t_wk[w, k]))]
        for w in range(NW)
    ]
    return eidx16, ev, el, batches, calls, win_cols, tc_total


def _build_program(batches, calls, win_cols, tc_total, edge_dtype):
    import concourse.bacc as bacc
    import concourse.tile as tile
    from concourse import mybir

    NW, NK = _derived()
    RPAD = NW * WIN
    f32 = mybir.dt.float32
    i16 = mybir.dt.int16
    dt_edge = {"f32": mybir.dt.float32, "bf16": mybir.dt.bfloat16}[edge_dtype]

    nc = bacc.Bacc("TRN2", target_bir_lowering=False, debug=False, num_swdge_queues=NSWQ)

    xin = nc.declare_dram_parameter("xin", [N_NODES, C], dt_edge, isOutput=False)
    eidx_d = nc.declare_dram_parameter("eidx", [128, tc_total * 8], i16, isOutput=False)
    ev_d = nc.declare_dram_parameter("ev", [128, tc_total], dt_edge, isOutput=False)
    el_d = nc.declare_dram_parameter("el", [128, tc_total], dt_edge, isOutput=False)
    need_f32 = ACT_FRAC > 0 or TSCOL
    if need_f32:
        evf_d = nc.declare_dram_parameter("evf", [128, tc_total], f32, isOutput=False)
        elf_d = nc.declare_dram_parameter("elf", [128, tc_total], f32, isOutput=False)
    wt_d = nc.declare_dram_parameter("wt", [C, C], f32, isOutput=False)
    bias_d = nc.declare_dram_parameter("bias", [1, C], f32, isOutput=False)
    ones_d = nc.declare_dram_parameter("ones", [1, WIN], f32, isOutput=False)
    iota_d = nc.declare_dram_parameter("iota", [WIN, KS * WIN], dt_edge, isOutput=False)
    iotar_d = nc.declare_dram_parameter("iotar", [128, WIN * KS], dt_edge, isOutput=False)
    out_d = nc.declare_dram_parameter("out", [RPAD, C], f32, isOutput=True)

    calls_by_batch = {}
    for bi, k, cb, ck in calls:
        calls_by_batch.setdefault(bi, []).append((k, cb, ck))

    with tile.TileContext(nc) as tc:
        with (
            tc.tile_pool(name="consts", bufs=1) as consts,
            tc.tile_pool(name="xgp", bufs=XG_BUFS) as xgp,
            tc.tile_pool(name="sp", bufs=SM_BUFS) as sp,
            tc.tile_pool(name="op", bufs=3) as op,
            tc.tile_pool(name="actp", bufs=4) as actp,
            tc.tile_pool(name="psum", bufs=PSUM_BUFS, space="PSUM") as psum,
        ):
            iota_t = consts.tile([WIN, KS * WIN], dt_edge)
            iotar_t = consts.tile([128, WIN * KS], dt_edge)
            wt_t = consts.tile([C, C], f32)
            bias_t = consts.tile([1, C], f32)
            ones_t = consts.tile([1, WIN], f32)
            split_col = batches[2][0] if len(batches) > 2 else None
            if split_col is not None:
                split_col = min(cb for _, cb, _ in calls_by_batch[2])
            else:
                split_col = tc_total
            eidx_a = consts.tile([128, split_col * 8], i16)
            eidx_b = consts.tile([128, (tc_total - split_col) * 8], i16)
            ev_t = consts.tile([128, tc_total], dt_edge)
            el_t = consts.tile([128, tc_total], dt_edge)
            if need_f32:
                evf_t = consts.tile([128, tc_total], f32)
                elf_t = consts.tile([128, tc_total], f32)
            nc.sync.dma_start(eidx_a[:], eidx_d[:, : split_col * 8])
            nc.sync.dma_start(ev_t[:], ev_d[:])
            nc.sync.dma_start(el_t[:], el_d[:])
            nc.sync.dma_start(iota_t[:], iota_d[:])
            nc.sync.dma_start(iotar_t[:], iotar_d[:])
            nc.sync.dma_start(wt_t[:], wt_d[:])
            nc.sync.dma_start(bias_t[:], bias_d[:])
            nc.sync.dma_start(ones_t[:], ones_d[:])
            if tc_total > split_col:
                nc.sync.dma_start(eidx_b[:], eidx_d[:, split_col * 8 :])
            if need_f32:
                nc.sync.dma_start(evf_t[:], evf_d[:])
                nc.sync.dma_start(elf_t[:], elf_d[:])

            qi = 0
            for bi, (w0, nwin, ncols) in enumerate(batches):
                c0 = min(cb for _, cb, _ in calls_by_batch[bi])

                xg = xgp.tile([128, CB, C], dt_edge, tag="xg")
                for k, cb, ck in calls_by_batch[bi]:
                    kb = CHUNK_BOUNDS[k]
                    rows_k = CHUNK_BOUNDS[k + 1] - kb
                    step = -(-ck // SUBCALLS)
                    o = cb
                    while o < cb + ck:
                        cs = min(step, cb + ck - o)
                        lb = o - c0
                        if o >= split_col:
                            eidx_ap = eidx_b[:, (o - split_col) * 8 : (o - split_col + cs) * 8]
                        else:
                            eidx_ap = eidx_a[:, o * 8 : (o + cs) * 8]
                        nc.gpsimd.dma_gather(
                            xg[:, lb : lb + cs, :],
                            xin[kb : kb + rows_k, :],
                            eidx_ap,
                            cs * 128,
                            cs * 128,
                            C,
                            single_packet=False,
                            queue_num=qi % NSWQ,
                        )
                        qi += 1
                        o += step

                sm = sp.tile([128, CB * WIN], dt_edge, tag="s")
                act_cols = int(ncols * ACT_FRAC)
                dve_cols = ncols - act_cols
                if TRS:
                    dve_cols = (dve_cols // KS) * KS
                    act_cols = ncols - dve_cols
                    for g in range(dve_cols // KS):
                        smv = sm[
                            :, g * KS * WIN : (g + 1) * KS * WIN
                        ].rearrange("p (i c) -> p i c", i=WIN, c=KS)
                        elb = el_t[
                            :, c0 + g * KS : c0 + (g + 1) * KS
                        ].unsqueeze(1).to_broadcast([128, WIN, KS])
                        evb = ev_t[
                            :, c0 + g * KS : c0 + (g + 1) * KS
                        ].unsqueeze(1).to_broadcast([128, WIN, KS])
                        nc.vector.tensor_tensor(
                            out=smv, in0=iotar_t[:].rearrange(
                                "p (i c) -> p i c", i=WIN, c=KS
                            ), in1=elb, op=mybir.AluOpType.is_equal,
                        )
                        nc.vector.tensor_tensor(
                            out=smv, in0=smv, in1=evb, op=mybir.AluOpType.mult,
                        )
                    grp_cols = 0
                elif TSCOL:
                    for lc in range(dve_cols):
                        nc.vector.tensor_scalar(
                            out=sm[:, lc * WIN : (lc + 1) * WIN],
                            in0=iota_t[:, :WIN],
                            scalar1=elf_t[:, c0 + lc : c0 + lc + 1],
                            scalar2=evf_t[:, c0 + lc : c0 + lc + 1],
                            op0=mybir.AluOpType.is_equal,
                            op1=mybir.AluOpType.mult,
                        )
                    grp_cols = 0
                else:
                    grp_cols = dve_cols
                for g in range(-(-grp_cols // KS)):
                    ncg = min(KS, grp_cols - g * KS)
                    smv = sm[:, g * KS * WIN : (g * KS + ncg) * WIN]
                    nc.vector.tensor_tensor(
                        out=smv,
                        in0=iota_t[:, : ncg * WIN],
                        in1=el_t[:, c0 + g * KS : c0 + g * KS + ncg].to_broadcast(
                            [128, ncg, WIN]
                        ),
                        op=mybir.AluOpType.is_equal,
                    )
                    nc.vector.tensor_tensor(
                        out=smv,
                        in0=smv,
                        in1=ev_t[:, c0 + g * KS : c0 + g * KS + ncg].to_broadcast(
                            [128, ncg, WIN]
                        ),
                        op=mybir.AluOpType.mult,
                    )
                # ScalarE builds the tail columns: Square(iota-lrow) ->
                # Relu(1-sq) -> *val  (exact for integer iota/lrow)
                for lc in range(dve_cols, ncols):
                    sq = actp.tile([128, WIN], dt_edge, tag="sq")
                    nc.scalar.activation(
                        sq[:],
                        iota_t[:, :WIN],
                        mybir.ActivationFunctionType.Square,
                        bias=elf_t[:, c0 + lc : c0 + lc + 1],
                        scale=-1.0,
                    )
                    oh = actp.tile([128, WIN], dt_edge, tag="oh")
                    nc.scalar.activation(
                        oh[:],
                        sq[:],
                        mybir.ActivationFunctionType.Relu,
                        bias=1.0,
                        scale=-1.0,
                    )
                    nc.scalar.mul(
                        sm[:, lc * WIN : (lc + 1) * WIN],
                        oh[:],
                        evf_t[:, c0 + lc : c0 + lc + 1],
                    )

                for w in range(w0, w0 + nwin):
                    wcols = win_cols[w]
                    psum1 = psum.tile([C, WIN], f32, tag="psum1")
                    for ti, col in enumerate(wcols):
                        lc = col - c0
                        if TRS and lc < (ncols - act_cols):
                            g, cc = lc // KS, lc % KS
                            rhs_ap = sm[
                                :, g * KS * WIN : (g + 1) * KS * WIN
                            ].rearrange("p (i c) -> p i c", i=WIN, c=KS)[:, :, cc]
                        else:
                            rhs_ap = sm[:, lc * WIN : (lc + 1) * WIN]
                        nc.tensor.matmul(
                            psum1[:],
                            lhsT=xg[:, lc, :],
                            rhs=rhs_ap,
                            start=(ti == 0),
                            stop=(ti == len(wcols) - 1),
                        )

                    # psum1 holds aggT [cin, rows]; out_w = aggT.T @ W.T + b
                    # (the bias lands via a rank-1 matmul; PSUM->SBUF copies
                    # run on the Scalar engine so DVE stays free for S builds)
                    aggT = op.tile([C, WIN], f32, tag="aggT")
                    nc.scalar.copy(aggT[:], psum1[:])
                    psum2 = psum.tile([WIN, C], f32, tag="psum2")
                    nc.tensor.matmul(
                        psum2[:], lhsT=aggT[:], rhs=wt_t[:], start=True, stop=False
                    )
                    nc.tensor.matmul(
                        psum2[:], lhsT=ones_t[:], rhs=bias_t[:], start=False, stop=True
                    )
                    outw = op.tile([WIN, C], f32, tag="outw")
                    nc.scalar.copy(outw[:], psum2[:])
                    nc.sync.dma_start(out_d[w * WIN : (w + 1) * WIN, :], outw[:])

    nc.compile()
    return nc


def kernel(x, rows, cols, vals, W, b):
    from concourse.bass_utils import run_bass_kernel_spmd

    NW, _ = _derived()
    x = np.ascontiguousarray(np.asarray(x), dtype=np.float32)
    W = np.asarray(W).astype(np.float32)
    b = np.asarray(b).astype(np.float32)

    eidx16, ev, el, batches, calls, win_cols, tc_total = _host_prep(rows, cols, vals)

    if EDGE_DTYPE == "bf16":
        import ml_dtypes

        x_dev = x.astype(ml_dtypes.bfloat16)
        mdt = ml_dtypes.bfloat16
    else:
        x_dev = x
        mdt = np.float32
    iota = np.ascontiguousarray(
        np.broadcast_to(
            np.tile(-np.arange(WIN, dtype=np.float32), KS), (WIN, KS * WIN)
        )
    ).astype(mdt)

    iota_rep = np.ascontiguousarray(
        np.broadcast_to(
            np.repeat(-np.arange(WIN, dtype=np.float32), KS), (128, WIN * KS)
        )
    ).astype(mdt)
    wt = np.ascontiguousarray(W.T)  # [cin, cout]
    bias_rep = np.ascontiguousarray(b.reshape(1, C))
    ones_row = np.ones((1, WIN), dtype=np.float32)

    nc = _build_program(batches, calls, win_cols, tc_total, EDGE_DTYPE)

    in_maps = [
        {
            "xin": x_dev,
            "eidx": np.ascontiguousarray(eidx16[c]),
            "ev": ev[c].astype(mdt),
            "el": el[c].astype(mdt),
            **({"evf": ev[c], "elf": el[c]} if (ACT_FRAC > 0 or TSCOL) else {}),
            "wt": wt,
            "bias": bias_rep,
            "ones": ones_row,
            "iota": np.ascontiguousarray(iota),
            "iotar": iota_rep,
        }
        for c in range(N_CORES)
    ]

    res = run_bass_kernel_spmd(nc, in_maps, list(range(N_CORES)), trace=TRACE)
    LAST_RESULT["exec_time_ns"] = res.exec_time_ns
    LAST_RESULT["results"] = res

    out = np.empty((N_NODES, C), dtype=np.float32)
    for c in range(N_CORES):
        out[c * RPC : (c + 1) * RPC] = res.results[c]["out"][:RPC]
    return out


# revision 30
# speedup vs baseline: 1.0270x; 1.0270x over previous
"""Trainium2 Bass kernel for nn_MeshConv (COO SpMM + 128x128 Linear).

out[r, :] = (sum_{e: rows[e]==r} vals[e] * x[cols[e], :]) @ W.T + b

Strategy (8 NeuronCores, one SPMD program):
  - Row-shard: core c owns output rows [c*12500, (c+1)*12500); x, W, b
    are replicated per core, so no collectives are needed.
  - Host packs each core's edges by (128-row output window, column
    chunk) into 128-edge tiles.  Chunk boundaries are tuned so the four
    per-batch gather calls are balanced and tile padding is minimal
    (each chunk <= 32768 rows for int16 gather indices).
  - ALL metadata planes (wrapped gather indices, vals, local rows) are
    SBUF-resident up front, so per-batch work has no metadata DMA
    dependency and gather drains run back-to-back.  Each chunk's gather
    is split into 2 sub-calls for finer SWDGE ring pipelining.
  - Per batch: dma_gather x rows (bf16, SWDGE, 4 queues) into SBUF;
    build the selection matrix S[e, r] = (iota_r == lrow_e) * val_e
    split across DVE (is_equal+mult on KS-column groups) and ScalarE
    (Square/Relu/mul per column, ACT_FRAC of columns); accumulate
    aggT[cin, rows] = X_tile^T @ S_tile per window in PSUM on TensorE.
  - Projection out_w = aggT.T @ W.T + bias: the bias lands via a rank-1
    matmul into the same PSUM bank and ScalarE does PSUM->SBUF copies,
    keeping DVE dedicated to S builds.
"""

import os
import sys

for _p in ("/opt/trn_rl_repo",):
    if _p not in sys.path:
        sys.path.insert(0, _p)

import numpy as np

# --- problem constants (from the problem spec) ---
N_NODES = 100000
C = 128
N_CORES = 8
RPC = N_NODES // N_CORES          # rows per core: 12500
WIN = 128                         # output window = PSUM partition dim
# column-chunk boundaries (each segment <= 32768 for int16 gather indices);
# sizes tuned to minimize 128-edge tile padding for this edge distribution
CHUNK_BOUNDS = [0, 28120, 55880, 72060, 100000]
CB = int(os.environ.get("MESHCONV_CB", "88"))

EDGE_DTYPE = os.environ.get("MESHCONV_EDGE_DTYPE", "bf16")
KS = 16                           # S-build tiles per DVE op
NSWQ = int(os.environ.get("MESHCONV_NSWQ", "4"))
XG_BUFS = int(os.environ.get("MESHCONV_XG_BUFS", "3"))
SM_BUFS = int(os.environ.get("MESHCONV_SM_BUFS", "3"))
PSUM_BUFS = int(os.environ.get("MESHCONV_PSUM_BUFS", "6"))
SUBCALLS = int(os.environ.get("MESHCONV_SUBCALLS", "2"))  # gather sub-calls per chunk
ACT_FRAC = float(os.environ.get("MESHCONV_ACT_FRAC", "0.23"))  # S-build share on ScalarE
TSCOL = os.environ.get("MESHCONV_TSCOL", "0") == "1"  # per-column tensor_scalar S build
TRS = os.environ.get("MESHCONV_TRS", "0") == "1"  # transposed S-group build (DVE 2x)

TRACE = False          # set by test.py for profiling runs
LAST_RESULT = {}       # test.py reads exec_time_ns etc. from here


def _derived():
    nw = (RPC + WIN - 1) // WIN
    nk = len(CHUNK_BOUNDS) - 1
    return nw, nk


def _host_prep(rows, cols, vals):
    """Pack edges per (core, window, chunk) into fixed 128-lane tiles."""
    NW, NK = _derived()
    rows = np.asarray(rows).astype(np.int64)
    cols = np.asarray(cols).astype(np.int64)
    vals = np.asarray(vals).astype(np.float32)

    core = rows // RPC
    lrow_full = rows - core * RPC
    win = lrow_full // WIN
    lrow = lrow_full - win * WIN
    bounds = np.asarray(CHUNK_BOUNDS)
    chunk = np.searchsorted(bounds, cols, side="right") - 1
    cidx = cols - bounds[chunk]

    gid = (core * NW + win) * NK + chunk
    cnt = np.bincount(gid, minlength=N_CORES * NW * NK).reshape(N_CORES, NW, NK)
    t_wk = -(-cnt.max(axis=0) // 128)         # [NW, NK]
    tw_tot = t_wk.sum(axis=1)
    for w in np.flatnonzero(tw_tot == 0):
        t_wk[w, 0] = 1
    tw_tot = t_wk.sum(axis=1)

    batches = []  # (w0, nwin, ncols)
    w = 0
    while w < NW:
        w0, ccols = w, 0
        while w < NW and ccols + tw_tot[w] <= CB:
            ccols += int(tw_tot[w])
            w += 1
        assert w > w0, f"window {w0} needs {tw_tot[w0]} > CB={CB} columns"
        batches.append((w0, w - w0, ccols))

    col_of = np.zeros((NW, NK), dtype=np.int64)
    calls = []  # (batch_idx, k, col_base, ncols)
    base = 0
    for bi, (w0, nwin, _) in enumerate(batches):
        for k in range(NK):
            cb = base
            for w in range(w0, w0 + nwin):
                col_of[w, k] = base
                base += int(t_wk[w, k])
            if base > cb:
                calls.append((bi, k, cb, base - cb))
    tc_total = int(base)

    order = np.lexsort((chunk, win, core))
    core_s, win_s, chunk_s = core[order], win[order], chunk[order]
    grp = (core_s * NW + win_s) * NK + chunk_s
    start_of_grp = np.searchsorted(grp, np.arange(N_CORES * NW * NK), side="left")
    rank = np.arange(len(grp)) - start_of_grp[grp]
    t = rank // 128
    p = rank - t * 128
    gcol = col_of[win_s, chunk_s] + t

    sidx = np.zeros((N_CORES, tc_total, 128), dtype=np.int16)
    sval = np.zeros((N_CORES, tc_total, 128), dtype=np.float32)
    slrow = np.zeros((N_CORES, tc_total, 128), dtype=np.float32)
    sidx[core_s, gcol, p] = cidx[order].astype(np.int16)
    sval[core_s, gcol, p] = vals[order]
    # stored negated: DVE compares against a negated iota plane, and the
    # ScalarE path uses it directly as the bias in Square(iota - lrow)
    slrow[core_s, gcol, p] = -lrow[order].astype(np.float32)

    # wrapped int16 index plane, per gather-call region (sub-call aware)
    eidx16 = np.zeros((N_CORES, 128, tc_total * 8), dtype=np.int16)
    sub_regions = []
    for _, _, cb, ck in calls:
        if SUBCALLS <= 1:
            sub_regions.append((cb, ck))
        else:
            step = -(-ck // SUBCALLS)
            o = cb
            while o < cb + ck:
                sub_regions.append((o, min(step, cb + ck - o)))
                o += step
    for cb, ck in sub_regions:
        flat = sidx[:, cb : cb + ck, :].reshape(N_CORES, ck * 128)
        wrapped = flat.reshape(N_CORES, ck * 8, 16).transpose(0, 2, 1)
        eidx16[:, :, cb * 8 : (cb + ck) * 8] = np.tile(wrapped, (1, 8, 1))

    ev = np.ascontiguousarray(sval.transpose(0, 2, 1))    # [NC, 128, TC]
    el = np.ascontiguousarray(slrow.transpose(0, 2, 1))

    win_cols = [
        [int(col_of[w, k]) + t for k in range(NK) for t in range(int(t_wk[w, k]))]
        for w in range(NW)
    ]
    return eidx16, ev, el, batches, calls, win_cols, tc_total


def _build_program(batches, calls, win_cols, tc_total, edge_dtype):
    import concourse.bacc as bacc
    import concourse.tile as tile
    from concourse import mybir

    NW, NK = _derived()
    RPAD = NW * WIN
    f32 = mybir.dt.float32
    i16 = mybir.dt.int16
    dt_edge = {"f32": mybir.dt.float32, "bf16": mybir.dt.bfloat16}[edge_dtype]

    nc = bacc.Bacc("TRN2", target_bir_lowering=False, debug=False, num_swdge_queues=NSWQ)

    xin = nc.declare_dram_parameter("xin", [N_NODES, C], dt_edge, isOutput=False)
    eidx_d = nc.declare_dram_parameter("eidx", [128, tc_total * 8], i16, isOutput=False)
    ev_d = nc.declare_dram_parameter("ev", [128, tc_total], dt_edge, isOutput=False)
    el_d = nc.declare_dram_parameter("el", [128, tc_total], dt_edge, isOutput=False)
    need_f32 = ACT_FRAC > 0 or TSCOL
    if need_f32:
        evf_d = nc.declare_dram_parameter("evf", [128, tc_total], f32, isOutput=False)
        elf_d = nc.declare_dram_parameter("elf", [128, tc_total], f32, isOutput=False)
    wt_d = nc.declare_dram_parameter("wt", [C, C], f32, isOutput=False)
    bias_d = nc.declare_dram_parameter("bias", [1, C], f32, isOutput=False)
    ones_d = nc.declare_dram_parameter("ones", [1, WIN], f32, isOutput=False)
    iota_d = nc.declare_dram_parameter("iota", [WIN, KS * WIN], dt_edge, isOutput=False)
    iotar_d = nc.declare_dram_parameter("iotar", [128, WIN * KS], dt_edge, isOutput=False)
    out_d = nc.declare_dram_parameter("out", [RPAD, C], f32, isOutput=True)

    calls_by_batch = {}
    for bi, k, cb, ck in calls:
        calls_by_batch.setdefault(bi, []).append((k, cb, ck))

    with tile.TileContext(nc) as tc:
        with (
            tc.tile_pool(name="consts", bufs=1) as consts,
            tc.tile_pool(name="xgp", bufs=XG_BUFS) as xgp,
            tc.tile_pool(name="sp", bufs=SM_BUFS) as sp,
            tc.tile_pool(name="op", bufs=3) as op,
            tc.tile_pool(name="actp", bufs=4) as actp,
            tc.tile_pool(name="psum", bufs=PSUM_BUFS, space="PSUM") as psum,
            tc.tile_pool(name="psumb", bufs=2, space="PSUM") as psumb,
        ):
            iota_t = consts.tile([WIN, KS * WIN], dt_edge)
            iotar_t = consts.tile([128, WIN * KS], dt_edge)
            wt_t = consts.tile([C, C], f32)
            bias_t = consts.tile([1, C], f32)
            ones_t = consts.tile([1, WIN], f32)
            split_col = batches[2][0] if len(batches) > 2 else None
            if split_col is not None:
                split_col = min(cb for _, cb, _ in calls_by_batch[2])
            else:
                split_col = tc_total
            eidx_a = consts.tile([128, split_col * 8], i16)
            eidx_b = consts.tile([128, (tc_total - split_col) * 8], i16)
            ev_t = consts.tile([128, tc_total], dt_edge)
            el_t = consts.tile([128, tc_total], dt_edge)
            if need_f32:
                evf_t = consts.tile([128, tc_total], f32)
                elf_t = consts.tile([128, tc_total], f32)
            nc.sync.dma_start(eidx_a[:], eidx_d[:, : split_col * 8])
            nc.sync.dma_start(ev_t[:], ev_d[:])
            nc.sync.dma_start(el_t[:], el_d[:])
            nc.sync.dma_start(iota_t[:], iota_d[:])
            nc.sync.dma_start(iotar_t[:], iotar_d[:])
            nc.sync.dma_start(wt_t[:], wt_d[:])
            nc.sync.dma_start(bias_t[:], bias_d[:])
            nc.sync.dma_start(ones_t[:], ones_d[:])
            if tc_total > split_col:
                nc.sync.dma_start(eidx_b[:], eidx_d[:, split_col * 8 :])
            if need_f32:
                nc.sync.dma_start(evf_t[:], evf_d[:])
                nc.sync.dma_start(elf_t[:], elf_d[:])

            qi = 0
            for bi, (w0, nwin, ncols) in enumerate(batches):
                c0 = min(cb for _, cb, _ in calls_by_batch[bi])

                xg = xgp.tile([128, CB, C], dt_edge, tag="xg")
                for k, cb, ck in calls_by_batch[bi]:
                    kb = CHUNK_BOUNDS[k]
                    rows_k = CHUNK_BOUNDS[k + 1] - kb
                    step = -(-ck // SUBCALLS)
                    o = cb
                    while o < cb + ck:
                        cs = min(step, cb + ck - o)
                        lb = o - c0
                        if o >= split_col:
                            eidx_ap = eidx_b[:, (o - split_col) * 8 : (o - split_col + cs) * 8]
                        else:
                            eidx_ap = eidx_a[:, o * 8 : (o + cs) * 8]
                        nc.gpsimd.dma_gather(
                            xg[:, lb : lb + cs, :],
                            xin[kb : kb + rows_k, :],
                            eidx_ap,
                            cs * 128,
                            cs * 128,
                            C,
                            single_packet=False,
                            queue_num=qi % NSWQ,
                        )
                        qi += 1
                        o += step

                sm = sp.tile([128, CB * WIN], dt_edge, tag="s")
                act_cols = int(ncols * ACT_FRAC)
                dve_cols = ncols - act_cols
                if TRS:
                    dve_cols = (dve_cols // KS) * KS
                    act_cols = ncols - dve_cols
                    for g in range(dve_cols // KS):
                        smv = sm[
                            :, g * KS * WIN : (g + 1) * KS * WIN
                        ].rearrange("p (i c) -> p i c", i=WIN, c=KS)
                        elb = el_t[
                            :, c0 + g * KS : c0 + (g + 1) * KS
                        ].unsqueeze(1).to_broadcast([128, WIN, KS])
                        evb = ev_t[
                            :, c0 + g * KS : c0 + (g + 1) * KS
                        ].unsqueeze(1).to_broadcast([128, WIN, KS])
                        nc.vector.tensor_tensor(
                            out=smv, in0=iotar_t[:].rearrange(
                                "p (i c) -> p i c", i=WIN, c=KS
                            ), in1=elb, op=mybir.AluOpType.is_equal,
                        )
                        nc.vector.tensor_tensor(
                            out=smv, in0=smv, in1=evb, op=mybir.AluOpType.mult,
                        )
                    grp_cols = 0
                elif TSCOL:
                    for lc in range(dve_cols):
                        nc.vector.tensor_scalar(
                            out=sm[:, lc * WIN : (lc + 1) * WIN],
                            in0=iota_t[:, :WIN],
                            scalar1=elf_t[:, c0 + lc : c0 + lc + 1],
                            scalar2=evf_t[:, c0 + lc : c0 + lc + 1],
                            op0=mybir.AluOpType.is_equal,
                            op1=mybir.AluOpType.mult,
                        )
                    grp_cols = 0
                else:
                    grp_cols = dve_cols
                for g in range(-(-grp_cols // KS)):
                    ncg = min(KS, grp_cols - g * KS)
                    smv = sm[:, g * KS * WIN : (g * KS + ncg) * WIN]
                    nc.vector.tensor_tensor(
                        out=smv,
                        in0=iota_t[:, : ncg * WIN],
                        in1=el_t[:, c0 + g * KS : c0 + g * KS + ncg].to_broadcast(
                            [128, ncg, WIN]
                        ),
                        op=mybir.AluOpType.is_equal,
                    )
                    nc.vector.tensor_tensor(
                        out=smv,
                        in0=smv,
                        in1=ev_t[:, c0 + g * KS : c0 + g * KS + ncg].to_broadcast(
                            [128, ncg, WIN]
                        ),
                        op=mybir.AluOpType.mult,
                    )
                # ScalarE builds the tail columns: Square(iota-lrow) ->
                # Relu(1-sq) -> *val  (exact for integer iota/lrow)
                for lc in range(dve_cols, ncols):
                    sq = actp.tile([128, WIN], dt_edge, tag="sq")
                    nc.scalar.activation(
                        sq[:],
                        iota_t[:, :WIN],
                        mybir.ActivationFunctionType.Square,
                        bias=elf_t[:, c0 + lc : c0 + lc + 1],
                        scale=-1.0,
                    )
                    oh = actp.tile([128, WIN], dt_edge, tag="oh")
                    nc.scalar.activation(
                        oh[:],
                        sq[:],
                        mybir.ActivationFunctionType.Relu,
                        bias=1.0,
                        scale=-1.0,
                    )
                    nc.scalar.mul(
                        sm[:, lc * WIN : (lc + 1) * WIN],
                        oh[:],
                        evf_t[:, c0 + lc : c0 + lc + 1],
                    )

                for w in range(w0, w0 + nwin):
                    wcols = win_cols[w]
                    psum1 = psum.tile([C, WIN], f32, tag="psum1")
                    for ti, col in enumerate(wcols):
                        lc = col - c0
                        if TRS and lc < (ncols - act_cols):
                            g, cc = lc // KS, lc % KS
                            rhs_ap = sm[
                                :, g * KS * WIN : (g + 1) * KS * WIN
                            ].rearrange("p (i c) -> p i c", i=WIN, c=KS)[:, :, cc]
                        else:
                            rhs_ap = sm[:, lc * WIN : (lc + 1) * WIN]
                        nc.tensor.matmul(
                            psum1[:],
                            lhsT=xg[:, lc, :],
                            rhs=rhs_ap,
                            start=(ti == 0),
                            stop=(ti == len(wcols) - 1),
                        )

                    # psum1 holds aggT [cin, rows]; out_w = aggT.T @ W.T + b
                    # (the bias lands via a rank-1 matmul; PSUM->SBUF copies
                    # run on the Scalar engine so DVE stays free for S builds)
                    aggT = op.tile([C, WIN], f32, tag="aggT")
                    nc.scalar.copy(aggT[:], psum1[:])
                    psum2 = psumb.tile([WIN, C], f32, tag="psum2")
                    nc.tensor.matmul(
                        psum2[:], lhsT=aggT[:], rhs=wt_t[:], start=True, stop=False
                    )
                    nc.tensor.matmul(
                        psum2[:], lhsT=ones_t[:], rhs=bias_t[:], start=False, stop=True
                    )
                    outw = op.tile([WIN, C], f32, tag="outw")
                    nc.scalar.copy(outw[:], psum2[:])
                    nc.sync.dma_start(out_d[w * WIN : (w + 1) * WIN, :], outw[:])

    nc.compile()
    return nc


def kernel(x, rows, cols, vals, W, b):
    from concourse.bass_utils import run_bass_kernel_spmd

    NW, _ = _derived()
    x = np.ascontiguousarray(np.asarray(x), dtype=np.float32)
    W = np.asarray(W).astype(np.float32)
    b = np.asarray(b).astype(np.float32)

    eidx16, ev, el, batches, calls, win_cols, tc_total = _host_prep(rows, cols, vals)

    if EDGE_DTYPE == "bf16":
        import ml_dtypes

        x_dev = x.astype(ml_dtypes.bfloat16)
        mdt = ml_dtypes.bfloat16
    else:
        x_dev = x
        mdt = np.float32
    iota = np.ascontiguousarray(
        np.broadcast_to(
            np.tile(-np.arange(WIN, dtype=np.float32), KS), (WIN, KS * WIN)
        )
    ).astype(mdt)

    iota_rep = np.ascontiguousarray(
        np.broadcast_to(
            np.repeat(-np.arange(WIN, dtype=np.float32), KS), (128, WIN * KS)
        )
    ).astype(mdt)
    wt = np.ascontiguousarray(W.T)  # [cin, cout]
    bias_rep = np.ascontiguousarray(b.reshape(1, C))
    ones_row = np.ones((1, WIN), dtype=np.float32)

    nc = _build_program(batches, calls, win_cols, tc_total, EDGE_DTYPE)

    in_maps = [
        {
            "xin": x_dev,
            "eidx": np.ascontiguousarray(eidx16[c]),
            "ev": ev[c].astype(mdt),
            "el": el[c].astype(mdt),
            **({"evf": ev[c], "elf": el[c]} if (ACT_FRAC > 0 or TSCOL) else {}),
            "wt": wt,
            "bias": bias_rep,
            "ones": ones_row,
            "iota": np.ascontiguousarray(iota),
            "iotar": iota_rep,
        }
        for c in range(N_CORES)
    ]

    res = run_bass_kernel_spmd(nc, in_maps, list(range(N_CORES)), trace=TRACE)
    LAST_RESULT["exec_time_ns"] = res.exec_time_ns
    LAST_RESULT["results"] = res

    out = np.empty((N_NODES, C), dtype=np.float32)
    for c in range(N_CORES):
        out[c * RPC : (c + 1) * RPC] = res.results[c]["out"][:RPC]
    return out


# revision 31
# speedup vs baseline: 1.0762x; 1.0480x over previous
"""Trainium2 Bass kernel for nn_MeshConv (COO SpMM + 128x128 Linear).

out[r, :] = (sum_{e: rows[e]==r} vals[e] * x[cols[e], :]) @ W.T + b

Strategy (8 NeuronCores, one SPMD program):
  - Row-shard: core c owns output rows [c*12500, (c+1)*12500); x, W, b
    are replicated per core, so no collectives are needed.
  - Host packs each core's edges by (128-row output window, column
    chunk) into 128-edge tiles.  Chunk boundaries are tuned so the four
    per-batch gather calls are balanced and tile padding is minimal
    (each chunk <= 32768 rows for int16 gather indices).
  - ALL metadata planes (wrapped gather indices, vals, local rows) are
    SBUF-resident up front, so per-batch work has no metadata DMA
    dependency and gather drains run back-to-back.  Each chunk's gather
    is split into 2 sub-calls for finer SWDGE ring pipelining.
  - Per batch: dma_gather x rows (bf16, SWDGE, 4 queues) into SBUF;
    build the selection matrix S[e, r] = (iota_r == lrow_e) * val_e
    split across DVE (is_equal+mult on KS-column groups) and ScalarE
    (Square/Relu/mul per column, ACT_FRAC of columns); accumulate
    aggT[cin, rows] = X_tile^T @ S_tile per window in PSUM on TensorE.
  - Projection out_w = aggT.T @ W.T + bias: the bias lands via a rank-1
    matmul into the same PSUM bank and ScalarE does PSUM->SBUF copies,
    keeping DVE dedicated to S builds.
"""

import os
import sys

for _p in ("/opt/trn_rl_repo",):
    if _p not in sys.path:
        sys.path.insert(0, _p)

import numpy as np

# --- problem constants (from the problem spec) ---
N_NODES = 100000
C = 128
N_CORES = 8
RPC = N_NODES // N_CORES          # rows per core: 12500
WIN = 128                         # output window = PSUM partition dim
# column-chunk boundaries (each segment <= 32768 for int16 gather indices);
# sizes tuned to minimize 128-edge tile padding for this edge distribution
CHUNK_BOUNDS = [0, 28120, 55880, 72060, 100000]
CB = int(os.environ.get("MESHCONV_CB", "88"))

EDGE_DTYPE = os.environ.get("MESHCONV_EDGE_DTYPE", "bf16")
KS = 16                           # S-build tiles per DVE op
NSWQ = int(os.environ.get("MESHCONV_NSWQ", "4"))
XG_BUFS = int(os.environ.get("MESHCONV_XG_BUFS", "3"))
SM_BUFS = int(os.environ.get("MESHCONV_SM_BUFS", "3"))
PSUM_BUFS = int(os.environ.get("MESHCONV_PSUM_BUFS", "4"))
SUBCALLS = int(os.environ.get("MESHCONV_SUBCALLS", "2"))  # gather sub-calls per chunk
ACT_FRAC = float(os.environ.get("MESHCONV_ACT_FRAC", "0.23"))  # S-build share on ScalarE
TSCOL = os.environ.get("MESHCONV_TSCOL", "0") == "1"  # per-column tensor_scalar S build
TRS = os.environ.get("MESHCONV_TRS", "0") == "1"  # transposed S-group build (DVE 2x)

TRACE = False          # set by test.py for profiling runs
LAST_RESULT = {}       # test.py reads exec_time_ns etc. from here


def _derived():
    nw = (RPC + WIN - 1) // WIN
    nk = len(CHUNK_BOUNDS) - 1
    return nw, nk


def _host_prep(rows, cols, vals):
    """Pack edges per (core, window, chunk) into fixed 128-lane tiles."""
    NW, NK = _derived()
    rows = np.asarray(rows).astype(np.int64)
    cols = np.asarray(cols).astype(np.int64)
    vals = np.asarray(vals).astype(np.float32)

    core = rows // RPC
    lrow_full = rows - core * RPC
    win = lrow_full // WIN
    lrow = lrow_full - win * WIN
    bounds = np.asarray(CHUNK_BOUNDS)
    chunk = np.searchsorted(bounds, cols, side="right") - 1
    cidx = cols - bounds[chunk]

    gid = (core * NW + win) * NK + chunk
    cnt = np.bincount(gid, minlength=N_CORES * NW * NK).reshape(N_CORES, NW, NK)
    t_wk = -(-cnt.max(axis=0) // 128)         # [NW, NK]
    tw_tot = t_wk.sum(axis=1)
    for w in np.flatnonzero(tw_tot == 0):
        t_wk[w, 0] = 1
    tw_tot = t_wk.sum(axis=1)

    batches = []  # (w0, nwin, ncols)
    w = 0
    while w < NW:
        w0, ccols = w, 0
        while w < NW and ccols + tw_tot[w] <= CB:
            ccols += int(tw_tot[w])
            w += 1
        assert w > w0, f"window {w0} needs {tw_tot[w0]} > CB={CB} columns"
        batches.append((w0, w - w0, ccols))

    col_of = np.zeros((NW, NK), dtype=np.int64)
    calls = []  # (batch_idx, k, col_base, ncols)
    base = 0
    for bi, (w0, nwin, _) in enumerate(batches):
        for k in range(NK):
            cb = base
            for w in range(w0, w0 + nwin):
                col_of[w, k] = base
                base += int(t_wk[w, k])
            if base > cb:
                calls.append((bi, k, cb, base - cb))
    tc_total = int(base)

    order = np.lexsort((chunk, win, core))
    core_s, win_s, chunk_s = core[order], win[order], chunk[order]
    grp = (core_s * NW + win_s) * NK + chunk_s
    start_of_grp = np.searchsorted(grp, np.arange(N_CORES * NW * NK), side="left")
    rank = np.arange(len(grp)) - start_of_grp[grp]
    t = rank // 128
    p = rank - t * 128
    gcol = col_of[win_s, chunk_s] + t

    sidx = np.zeros((N_CORES, tc_total, 128), dtype=np.int16)
    sval = np.zeros((N_CORES, tc_total, 128), dtype=np.float32)
    slrow = np.zeros((N_CORES, tc_total, 128), dtype=np.float32)
    sidx[core_s, gcol, p] = cidx[order].astype(np.int16)
    sval[core_s, gcol, p] = vals[order]
    # stored negated: DVE compares against a negated iota plane, and the
    # ScalarE path uses it directly as the bias in Square(iota - lrow)
    slrow[core_s, gcol, p] = -lrow[order].astype(np.float32)

    # wrapped int16 index plane, per gather-call region (sub-call aware)
    eidx16 = np.zeros((N_CORES, 128, tc_total * 8), dtype=np.int16)
    sub_regions = []
    for _, _, cb, ck in calls:
        if SUBCALLS <= 1:
            sub_regions.append((cb, ck))
        else:
            step = -(-ck // SUBCALLS)
            o = cb
            while o < cb + ck:
                sub_regions.append((o, min(step, cb + ck - o)))
                o += step
    for cb, ck in sub_regions:
        flat = sidx[:, cb : cb + ck, :].reshape(N_CORES, ck * 128)
        wrapped = flat.reshape(N_CORES, ck * 8, 16).transpose(0, 2, 1)
        eidx16[:, :, cb * 8 : (cb + ck) * 8] = np.tile(wrapped, (1, 8, 1))

    ev = np.ascontiguousarray(sval.transpose(0, 2, 1))    # [NC, 128, TC]
    el = np.ascontiguousarray(slrow.transpose(0, 2, 1))

    win_cols = [
        [int(col_of[w, k]) + t for k in range(NK) for t in range(int(t_wk[w, k]))]
        for w in range(NW)
    ]
    return eidx16, ev, el, batches, calls, win_cols, tc_total


def _build_program(batches, calls, win_cols, tc_total, edge_dtype):
    import concourse.bacc as bacc
    import concourse.tile as tile
    from concourse import mybir

    NW, NK = _derived()
    RPAD = NW * WIN
    f32 = mybir.dt.float32
    i16 = mybir.dt.int16
    dt_edge = {"f32": mybir.dt.float32, "bf16": mybir.dt.bfloat16}[edge_dtype]

    nc = bacc.Bacc("TRN2", target_bir_lowering=False, debug=False, num_swdge_queues=NSWQ)

    xin = nc.declare_dram_parameter("xin", [N_NODES, C], dt_edge, isOutput=False)
    eidx_d = nc.declare_dram_parameter("eidx", [128, tc_total * 8], i16, isOutput=False)
    ev_d = nc.declare_dram_parameter("ev", [128, tc_total], dt_edge, isOutput=False)
    el_d = nc.declare_dram_parameter("el", [128, tc_total], dt_edge, isOutput=False)
    need_f32 = ACT_FRAC > 0 or TSCOL
    if need_f32:
        evf_d = nc.declare_dram_parameter("evf", [128, tc_total], f32, isOutput=False)
        elf_d = nc.declare_dram_parameter("elf", [128, tc_total], f32, isOutput=False)
    wt_d = nc.declare_dram_parameter("wt", [C, C], f32, isOutput=False)
    bias_d = nc.declare_dram_parameter("bias", [1, C], f32, isOutput=False)
    ones_d = nc.declare_dram_parameter("ones", [1, WIN], f32, isOutput=False)
    iota_d = nc.declare_dram_parameter("iota", [WIN, KS * WIN], dt_edge, isOutput=False)
    iotar_d = nc.declare_dram_parameter("iotar", [128, WIN * KS], dt_edge, isOutput=False)
    out_d = nc.declare_dram_parameter("out", [RPAD, C], f32, isOutput=True)

    calls_by_batch = {}
    for bi, k, cb, ck in calls:
        calls_by_batch.setdefault(bi, []).append((k, cb, ck))

    with tile.TileContext(nc) as tc:
        with (
            tc.tile_pool(name="consts", bufs=1) as consts,
            tc.tile_pool(name="xgp", bufs=XG_BUFS) as xgp,
            tc.tile_pool(name="sp", bufs=SM_BUFS) as sp,
            tc.tile_pool(name="op", bufs=3) as op,
            tc.tile_pool(name="actp", bufs=4) as actp,
            tc.tile_pool(name="psum", bufs=PSUM_BUFS, space="PSUM") as psum,
        ):
            iota_t = consts.tile([WIN, KS * WIN], dt_edge)
            iotar_t = consts.tile([128, WIN * KS], dt_edge)
            wt_t = consts.tile([C, C], f32)
            bias_t = consts.tile([1, C], f32)
            ones_t = consts.tile([1, WIN], f32)
            split_col = batches[2][0] if len(batches) > 2 else None
            if split_col is not None:
                split_col = min(cb for _, cb, _ in calls_by_batch[2])
            else:
                split_col = tc_total
            eidx_a = consts.tile([128, split_col * 8], i16)
            eidx_b = consts.tile([128, (tc_total - split_col) * 8], i16)
            ev_t = consts.tile([128, tc_total], dt_edge)
            el_t = consts.tile([128, tc_total], dt_edge)
            if need_f32:
                evf_t = consts.tile([128, tc_total], f32)
                elf_t = consts.tile([128, tc_total], f32)
            nc.sync.dma_start(eidx_a[:], eidx_d[:, : split_col * 8])
            nc.sync.dma_start(ev_t[:], ev_d[:])
            nc.sync.dma_start(el_t[:], el_d[:])
            nc.sync.dma_start(iota_t[:], iota_d[:])
            nc.sync.dma_start(iotar_t[:], iotar_d[:])
            nc.sync.dma_start(wt_t[:], wt_d[:])
            nc.sync.dma_start(bias_t[:], bias_d[:])
            nc.sync.dma_start(ones_t[:], ones_d[:])
            if tc_total > split_col:
                nc.sync.dma_start(eidx_b[:], eidx_d[:, split_col * 8 :])
            if need_f32:
                nc.sync.dma_start(evf_t[:], evf_d[:])
                nc.sync.dma_start(elf_t[:], elf_d[:])

            qi = 0
            for bi, (w0, nwin, ncols) in enumerate(batches):
                c0 = min(cb for _, cb, _ in calls_by_batch[bi])

                xg = xgp.tile([128, CB, C], dt_edge, tag="xg")
                for k, cb, ck in calls_by_batch[bi]:
                    kb = CHUNK_BOUNDS[k]
                    rows_k = CHUNK_BOUNDS[k + 1] - kb
                    step = -(-ck // SUBCALLS)
                    o = cb
                    while o < cb + ck:
                        cs = min(step, cb + ck - o)
                        lb = o - c0
                        if o >= split_col:
                            eidx_ap = eidx_b[:, (o - split_col) * 8 : (o - split_col + cs) * 8]
                        else:
                            eidx_ap = eidx_a[:, o * 8 : (o + cs) * 8]
                        nc.gpsimd.dma_gather(
                            xg[:, lb : lb + cs, :],
                            xin[kb : kb + rows_k, :],
                            eidx_ap,
                            cs * 128,
                            cs * 128,
                            C,
                            single_packet=False,
                            queue_num=qi % NSWQ,
                        )
                        qi += 1
                        o += step

                sm = sp.tile([128, CB * WIN], dt_edge, tag="s")
                act_cols = int(ncols * ACT_FRAC)
                dve_cols = ncols - act_cols
                if TRS:
                    dve_cols = (dve_cols // KS) * KS
                    act_cols = ncols - dve_cols
                    for g in range(dve_cols // KS):
                        smv = sm[
                            :, g * KS * WIN : (g + 1) * KS * WIN
                        ].rearrange("p (i c) -> p i c", i=WIN, c=KS)
                        elb = el_t[
                            :, c0 + g * KS : c0 + (g + 1) * KS
                        ].unsqueeze(1).to_broadcast([128, WIN, KS])
                        evb = ev_t[
                            :, c0 + g * KS : c0 + (g + 1) * KS
                        ].unsqueeze(1).to_broadcast([128, WIN, KS])
                        nc.vector.tensor_tensor(
                            out=smv, in0=iotar_t[:].rearrange(
                                "p (i c) -> p i c", i=WIN, c=KS
                            ), in1=elb, op=mybir.AluOpType.is_equal,
                        )
                        nc.vector.tensor_tensor(
                            out=smv, in0=smv, in1=evb, op=mybir.AluOpType.mult,
                        )
                    grp_cols = 0
                elif TSCOL:
                    for lc in range(dve_cols):
                        nc.vector.tensor_scalar(
                            out=sm[:, lc * WIN : (lc + 1) * WIN],
                            in0=iota_t[:, :WIN],
                            scalar1=elf_t[:, c0 + lc : c0 + lc + 1],
                            scalar2=evf_t[:, c0 + lc : c0 + lc + 1],
                            op0=mybir.AluOpType.is_equal,
                            op1=mybir.AluOpType.mult,
                        )
                    grp_cols = 0
                else:
                    grp_cols = dve_cols
                for g in range(-(-grp_cols // KS)):
                    ncg = min(KS, grp_cols - g * KS)
                    smv = sm[:, g * KS * WIN : (g * KS + ncg) * WIN]
                    nc.vector.tensor_tensor(
                        out=smv,
                        in0=iota_t[:, : ncg * WIN],
                        in1=el_t[:, c0 + g * KS : c0 + g * KS + ncg].to_broadcast(
                            [128, ncg, WIN]
                        ),
                        op=mybir.AluOpType.is_equal,
                    )
                    nc.vector.tensor_tensor(
                        out=smv,
                        in0=smv,
                        in1=ev_t[:, c0 + g * KS : c0 + g * KS + ncg].to_broadcast(
                            [128, ncg, WIN]
                        ),
                        op=mybir.AluOpType.mult,
                    )
                # ScalarE builds the tail columns: Square(iota-lrow) ->
                # Relu(1-sq) -> *val  (exact for integer iota/lrow)
                for lc in range(dve_cols, ncols):
                    sq = actp.tile([128, WIN], dt_edge, tag="sq")
                    nc.scalar.activation(
                        sq[:],
                        iota_t[:, :WIN],
                        mybir.ActivationFunctionType.Square,
                        bias=elf_t[:, c0 + lc : c0 + lc + 1],
                        scale=-1.0,
                    )
                    oh = actp.tile([128, WIN], dt_edge, tag="oh")
                    nc.scalar.activation(
                        oh[:],
                        sq[:],
                        mybir.ActivationFunctionType.Relu,
                        bias=1.0,
                        scale=-1.0,
                    )
                    nc.scalar.mul(
                        sm[:, lc * WIN : (lc + 1) * WIN],
                        oh[:],
                        evf_t[:, c0 + lc : c0 + lc + 1],
                    )

                for w in range(w0, w0 + nwin):
                    wcols = win_cols[w]
                    psum1 = psum.tile([C, WIN], f32, tag="psum1")
                    for ti, col in enumerate(wcols):
                        lc = col - c0
                        if TRS and lc < (ncols - act_cols):
                            g, cc = lc // KS, lc % KS
                            rhs_ap = sm[
                                :, g * KS * WIN : (g + 1) * KS * WIN
                            ].rearrange("p (i c) -> p i c", i=WIN, c=KS)[:, :, cc]
                        else:
                            rhs_ap = sm[:, lc * WIN : (lc + 1) * WIN]
                        nc.tensor.matmul(
                            psum1[:],
                            lhsT=xg[:, lc, :],
                            rhs=rhs_ap,
                            start=(ti == 0),
                            stop=(ti == len(wcols) - 1),
                        )

                    # psum1 holds aggT [cin, rows]; out_w = aggT.T @ W.T + b
                    # (the bias lands via a rank-1 matmul; PSUM->SBUF copies
                    # run on the Scalar engine so DVE stays free for S builds)
                    aggT = op.tile([C, WIN], f32, tag="aggT")
                    nc.scalar.copy(aggT[:], psum1[:])
                    psum2 = psum.tile([WIN, C], f32, tag="psum2")
                    nc.tensor.matmul(
                        psum2[:], lhsT=aggT[:], rhs=wt_t[:], start=True, stop=False
                    )
                    nc.tensor.matmul(
                        psum2[:], lhsT=ones_t[:], rhs=bias_t[:], start=False, stop=True
                    )
                    outw = op.tile([WIN, C], f32, tag="outw")
                    nc.scalar.copy(outw[:], psum2[:])
                    nc.sync.dma_start(out_d[w * WIN : (w + 1) * WIN, :], outw[:])

    nc.compile()
    return nc


def kernel(x, rows, cols, vals, W, b):
    from concourse.bass_utils import run_bass_kernel_spmd

    NW, _ = _derived()
    x = np.ascontiguousarray(np.asarray(x), dtype=np.float32)
    W = np.asarray(W).astype(np.float32)
    b = np.asarray(b).astype(np.float32)

    eidx16, ev, el, batches, calls, win_cols, tc_total = _host_prep(rows, cols, vals)

    if EDGE_DTYPE == "bf16":
        import ml_dtypes

        x_dev = x.astype(ml_dtypes.bfloat16)
        mdt = ml_dtypes.bfloat16
    else:
        x_dev = x
        mdt = np.float32
    iota = np.ascontiguousarray(
        np.broadcast_to(
            np.tile(-np.arange(WIN, dtype=np.float32), KS), (WIN, KS * WIN)
        )
    ).astype(mdt)

    iota_rep = np.ascontiguousarray(
        np.broadcast_to(
            np.repeat(-np.arange(WIN, dtype=np.float32), KS), (128, WIN * KS)
        )
    ).astype(mdt)
    wt = np.ascontiguousarray(W.T)  # [cin, cout]
    bias_rep = np.ascontiguousarray(b.reshape(1, C))
    ones_row = np.ones((1, WIN), dtype=np.float32)

    nc = _build_program(batches, calls, win_cols, tc_total, EDGE_DTYPE)

    in_maps = [
        {
            "xin": x_dev,
            "eidx": np.ascontiguousarray(eidx16[c]),
            "ev": ev[c].astype(mdt),
            "el": el[c].astype(mdt),
            **({"evf": ev[c], "elf": el[c]} if (ACT_FRAC > 0 or TSCOL) else {}),
            "wt": wt,
            "bias": bias_rep,
            "ones": ones_row,
            "iota": np.ascontiguousarray(iota),
            "iotar": iota_rep,
        }
        for c in range(N_CORES)
    ]

    res = run_bass_kernel_spmd(nc, in_maps, list(range(N_CORES)), trace=TRACE)
    LAST_RESULT["exec_time_ns"] = res.exec_time_ns
    LAST_RESULT["results"] = res

    out = np.empty((N_NODES, C), dtype=np.float32)
    for c in range(N_CORES):
        out[c * RPC : (c + 1) * RPC] = res.results[c]["out"][:RPC]
    return out


# revision 32
# speedup vs baseline: 1.0831x; 1.0064x over previous
"""Trainium2 Bass kernel for nn_MeshConv (COO SpMM + 128x128 Linear).

out[r, :] = (sum_{e: rows[e]==r} vals[e] * x[cols[e], :]) @ W.T + b

Strategy (8 NeuronCores, one SPMD program):
  - Row-shard: core c owns output rows [c*12500, (c+1)*12500); x, W, b
    are replicated per core, so no collectives are needed.
  - Host packs each core's edges by (128-row output window, column
    chunk) into 128-edge tiles.  Chunk boundaries are tuned so the four
    per-batch gather calls are balanced and tile padding is minimal
    (each chunk <= 32768 rows for int16 gather indices).
  - ALL metadata planes (wrapped gather indices, vals, local rows) are
    SBUF-resident up front, so per-batch work has no metadata DMA
    dependency and gather drains run back-to-back.  Each chunk's gather
    is split into 2 sub-calls for finer SWDGE ring pipelining.
  - Per batch: dma_gather x rows (bf16, SWDGE, 4 queues) into SBUF;
    build the selection matrix S[e, r] = (iota_r == lrow_e) * val_e
    split across DVE (is_equal+mult on KS-column groups) and ScalarE
    (Square/Relu/mul per column, ACT_FRAC of columns); accumulate
    aggT[cin, rows] = X_tile^T @ S_tile per window in PSUM on TensorE.
  - Projection out_w = aggT.T @ W.T + bias: the bias lands via a rank-1
    matmul into the same PSUM bank and ScalarE does PSUM->SBUF copies,
    keeping DVE dedicated to S builds.
"""

import os
import sys

for _p in ("/opt/trn_rl_repo",):
    if _p not in sys.path:
        sys.path.insert(0, _p)

import numpy as np

# --- problem constants (from the problem spec) ---
N_NODES = 100000
C = 128
N_CORES = 8
RPC = N_NODES // N_CORES          # rows per core: 12500
WIN = 128                         # output window = PSUM partition dim
# column-chunk boundaries (each segment <= 32768 for int16 gather indices);
# sizes tuned to minimize 128-edge tile padding for this edge distribution
CHUNK_BOUNDS = [0, 28120, 55880, 72060, 100000]
CB = int(os.environ.get("MESHCONV_CB", "88"))

EDGE_DTYPE = os.environ.get("MESHCONV_EDGE_DTYPE", "bf16")
KS = 16                           # S-build tiles per DVE op
NSWQ = int(os.environ.get("MESHCONV_NSWQ", "4"))
XG_BUFS = int(os.environ.get("MESHCONV_XG_BUFS", "3"))
SM_BUFS = int(os.environ.get("MESHCONV_SM_BUFS", "3"))
PSUM_BUFS = int(os.environ.get("MESHCONV_PSUM_BUFS", "4"))
SUBCALLS = int(os.environ.get("MESHCONV_SUBCALLS", "2"))  # gather sub-calls per chunk
ACT_FRAC = float(os.environ.get("MESHCONV_ACT_FRAC", "0.23"))  # S-build share on ScalarE
TSCOL = os.environ.get("MESHCONV_TSCOL", "0") == "1"  # per-column tensor_scalar S build
TRS = os.environ.get("MESHCONV_TRS", "0") == "1"  # transposed S-group build (DVE 2x)

TRACE = False          # set by test.py for profiling runs
LAST_RESULT = {}       # test.py reads exec_time_ns etc. from here


def _derived():
    nw = (RPC + WIN - 1) // WIN
    nk = len(CHUNK_BOUNDS) - 1
    return nw, nk


def _host_prep(rows, cols, vals):
    """Pack edges per (core, window, chunk) into fixed 128-lane tiles."""
    NW, NK = _derived()
    rows = np.asarray(rows).astype(np.int64)
    cols = np.asarray(cols).astype(np.int64)
    vals = np.asarray(vals).astype(np.float32)

    core = rows // RPC
    lrow_full = rows - core * RPC
    win = lrow_full // WIN
    lrow = lrow_full - win * WIN
    bounds = np.asarray(CHUNK_BOUNDS)
    chunk = np.searchsorted(bounds, cols, side="right") - 1
    cidx = cols - bounds[chunk]

    gid = (core * NW + win) * NK + chunk
    cnt = np.bincount(gid, minlength=N_CORES * NW * NK).reshape(N_CORES, NW, NK)
    t_wk = -(-cnt.max(axis=0) // 128)         # [NW, NK]
    tw_tot = t_wk.sum(axis=1)
    for w in np.flatnonzero(tw_tot == 0):
        t_wk[w, 0] = 1
    tw_tot = t_wk.sum(axis=1)

    batches = []  # (w0, nwin, ncols)
    w = 0
    while w < NW:
        w0, ccols = w, 0
        while w < NW and ccols + tw_tot[w] <= CB:
            ccols += int(tw_tot[w])
            w += 1
        assert w > w0, f"window {w0} needs {tw_tot[w0]} > CB={CB} columns"
        batches.append((w0, w - w0, ccols))

    col_of = np.zeros((NW, NK), dtype=np.int64)
    calls = []  # (batch_idx, k, col_base, ncols)
    base = 0
    for bi, (w0, nwin, _) in enumerate(batches):
        for k in range(NK):
            cb = base
            for w in range(w0, w0 + nwin):
                col_of[w, k] = base
                base += int(t_wk[w, k])
            if base > cb:
                calls.append((bi, k, cb, base - cb))
    tc_total = int(base)

    # fastest key cidx: ascending gather addresses within each (w,k) cell
    # (HBM/bank-friendly descriptor order; the S matrix absorbs the slot
    # permutation at zero device cost)
    order = np.lexsort((cidx, chunk, win, core))
    core_s, win_s, chunk_s = core[order], win[order], chunk[order]
    grp = (core_s * NW + win_s) * NK + chunk_s
    start_of_grp = np.searchsorted(grp, np.arange(N_CORES * NW * NK), side="left")
    rank = np.arange(len(grp)) - start_of_grp[grp]
    t = rank // 128
    p = rank - t * 128
    gcol = col_of[win_s, chunk_s] + t

    sidx = np.zeros((N_CORES, tc_total, 128), dtype=np.int16)
    sval = np.zeros((N_CORES, tc_total, 128), dtype=np.float32)
    slrow = np.zeros((N_CORES, tc_total, 128), dtype=np.float32)
    sidx[core_s, gcol, p] = cidx[order].astype(np.int16)
    sval[core_s, gcol, p] = vals[order]
    # stored negated: DVE compares against a negated iota plane, and the
    # ScalarE path uses it directly as the bias in Square(iota - lrow)
    slrow[core_s, gcol, p] = -lrow[order].astype(np.float32)

    # wrapped int16 index plane, per gather-call region (sub-call aware)
    eidx16 = np.zeros((N_CORES, 128, tc_total * 8), dtype=np.int16)
    sub_regions = []
    for _, _, cb, ck in calls:
        if SUBCALLS <= 1:
            sub_regions.append((cb, ck))
        else:
            step = -(-ck // SUBCALLS)
            o = cb
            while o < cb + ck:
                sub_regions.append((o, min(step, cb + ck - o)))
                o += step
    for cb, ck in sub_regions:
        flat = sidx[:, cb : cb + ck, :].reshape(N_CORES, ck * 128)
        wrapped = flat.reshape(N_CORES, ck * 8, 16).transpose(0, 2, 1)
        eidx16[:, :, cb * 8 : (cb + ck) * 8] = np.tile(wrapped, (1, 8, 1))

    ev = np.ascontiguousarray(sval.transpose(0, 2, 1))    # [NC, 128, TC]
    el = np.ascontiguousarray(slrow.transpose(0, 2, 1))

    win_cols = [
        [int(col_of[w, k]) + t for k in range(NK) for t in range(int(t_wk[w, k]))]
        for w in range(NW)
    ]
    return eidx16, ev, el, batches, calls, win_cols, tc_total


def _build_program(batches, calls, win_cols, tc_total, edge_dtype):
    import concourse.bacc as bacc
    import concourse.tile as tile
    from concourse import mybir

    NW, NK = _derived()
    RPAD = NW * WIN
    f32 = mybir.dt.float32
    i16 = mybir.dt.int16
    dt_edge = {"f32": mybir.dt.float32, "bf16": mybir.dt.bfloat16}[edge_dtype]

    nc = bacc.Bacc("TRN2", target_bir_lowering=False, debug=False, num_swdge_queues=NSWQ)

    xin = nc.declare_dram_parameter("xin", [N_NODES, C], dt_edge, isOutput=False)
    eidx_d = nc.declare_dram_parameter("eidx", [128, tc_total * 8], i16, isOutput=False)
    ev_d = nc.declare_dram_parameter("ev", [128, tc_total], dt_edge, isOutput=False)
    el_d = nc.declare_dram_parameter("el", [128, tc_total], dt_edge, isOutput=False)
    need_f32 = ACT_FRAC > 0 or TSCOL
    if need_f32:
        evf_d = nc.declare_dram_parameter("evf", [128, tc_total], f32, isOutput=False)
        elf_d = nc.declare_dram_parameter("elf", [128, tc_total], f32, isOutput=False)
    wt_d = nc.declare_dram_parameter("wt", [C, C], f32, isOutput=False)
    bias_d = nc.declare_dram_parameter("bias", [1, C], f32, isOutput=False)
    ones_d = nc.declare_dram_parameter("ones", [1, WIN], f32, isOutput=False)
    iota_d = nc.declare_dram_parameter("iota", [WIN, KS * WIN], dt_edge, isOutput=False)
    iotar_d = nc.declare_dram_parameter("iotar", [128, WIN * KS], dt_edge, isOutput=False)
    out_d = nc.declare_dram_parameter("out", [RPAD, C], f32, isOutput=True)

    calls_by_batch = {}
    for bi, k, cb, ck in calls:
        calls_by_batch.setdefault(bi, []).append((k, cb, ck))

    with tile.TileContext(nc) as tc:
        with (
            tc.tile_pool(name="consts", bufs=1) as consts,
            tc.tile_pool(name="xgp", bufs=XG_BUFS) as xgp,
            tc.tile_pool(name="sp", bufs=SM_BUFS) as sp,
            tc.tile_pool(name="op", bufs=3) as op,
            tc.tile_pool(name="actp", bufs=4) as actp,
            tc.tile_pool(name="psum", bufs=PSUM_BUFS, space="PSUM") as psum,
        ):
            iota_t = consts.tile([WIN, KS * WIN], dt_edge)
            iotar_t = consts.tile([128, WIN * KS], dt_edge)
            wt_t = consts.tile([C, C], f32)
            bias_t = consts.tile([1, C], f32)
            ones_t = consts.tile([1, WIN], f32)
            split_col = batches[2][0] if len(batches) > 2 else None
            if split_col is not None:
                split_col = min(cb for _, cb, _ in calls_by_batch[2])
            else:
                split_col = tc_total
            eidx_a = consts.tile([128, split_col * 8], i16)
            eidx_b = consts.tile([128, (tc_total - split_col) * 8], i16)
            ev_t = consts.tile([128, tc_total], dt_edge)
            el_t = consts.tile([128, tc_total], dt_edge)
            if need_f32:
                evf_t = consts.tile([128, tc_total], f32)
                elf_t = consts.tile([128, tc_total], f32)
            nc.sync.dma_start(eidx_a[:], eidx_d[:, : split_col * 8])
            nc.sync.dma_start(ev_t[:], ev_d[:])
            nc.sync.dma_start(el_t[:], el_d[:])
            nc.sync.dma_start(iota_t[:], iota_d[:])
            nc.sync.dma_start(iotar_t[:], iotar_d[:])
            nc.sync.dma_start(wt_t[:], wt_d[:])
            nc.sync.dma_start(bias_t[:], bias_d[:])
            nc.sync.dma_start(ones_t[:], ones_d[:])
            if tc_total > split_col:
                nc.sync.dma_start(eidx_b[:], eidx_d[:, split_col * 8 :])
            if need_f32:
                nc.sync.dma_start(evf_t[:], evf_d[:])
                nc.sync.dma_start(elf_t[:], elf_d[:])

            qi = 0
            for bi, (w0, nwin, ncols) in enumerate(batches):
                c0 = min(cb for _, cb, _ in calls_by_batch[bi])

                xg = xgp.tile([128, CB, C], dt_edge, tag="xg")
                for k, cb, ck in calls_by_batch[bi]:
                    kb = CHUNK_BOUNDS[k]
                    rows_k = CHUNK_BOUNDS[k + 1] - kb
                    step = -(-ck // SUBCALLS)
                    o = cb
                    while o < cb + ck:
                        cs = min(step, cb + ck - o)
                        lb = o - c0
                        if o >= split_col:
                            eidx_ap = eidx_b[:, (o - split_col) * 8 : (o - split_col + cs) * 8]
                        else:
                            eidx_ap = eidx_a[:, o * 8 : (o + cs) * 8]
                        nc.gpsimd.dma_gather(
                            xg[:, lb : lb + cs, :],
                            xin[kb : kb + rows_k, :],
                            eidx_ap,
                            cs * 128,
                            cs * 128,
                            C,
                            single_packet=False,
                            queue_num=qi % NSWQ,
                        )
                        qi += 1
                        o += step

                sm = sp.tile([128, CB * WIN], dt_edge, tag="s")
                act_cols = int(ncols * ACT_FRAC)
                dve_cols = ncols - act_cols
                if TRS:
                    dve_cols = (dve_cols // KS) * KS
                    act_cols = ncols - dve_cols
                    for g in range(dve_cols // KS):
                        smv = sm[
                            :, g * KS * WIN : (g + 1) * KS * WIN
                        ].rearrange("p (i c) -> p i c", i=WIN, c=KS)
                        elb = el_t[
                            :, c0 + g * KS : c0 + (g + 1) * KS
                        ].unsqueeze(1).to_broadcast([128, WIN, KS])
                        evb = ev_t[
                            :, c0 + g * KS : c0 + (g + 1) * KS
                        ].unsqueeze(1).to_broadcast([128, WIN, KS])
                        nc.vector.tensor_tensor(
                            out=smv, in0=iotar_t[:].rearrange(
                                "p (i c) -> p i c", i=WIN, c=KS
                            ), in1=elb, op=mybir.AluOpType.is_equal,
                        )
                        nc.vector.tensor_tensor(
                            out=smv, in0=smv, in1=evb, op=mybir.AluOpType.mult,
                        )
                    grp_cols = 0
                elif TSCOL:
                    for lc in range(dve_cols):
                        nc.vector.tensor_scalar(
                            out=sm[:, lc * WIN : (lc + 1) * WIN],
                            in0=iota_t[:, :WIN],
                            scalar1=elf_t[:, c0 + lc : c0 + lc + 1],
                            scalar2=evf_t[:, c0 + lc : c0 + lc + 1],
                            op0=mybir.AluOpType.is_equal,
                            op1=mybir.AluOpType.mult,
                        )
                    grp_cols = 0
                else:
                    grp_cols = dve_cols
                for g in range(-(-grp_cols // KS)):
                    ncg = min(KS, grp_cols - g * KS)
                    smv = sm[:, g * KS * WIN : (g * KS + ncg) * WIN]
                    nc.vector.tensor_tensor(
                        out=smv,
                        in0=iota_t[:, : ncg * WIN],
                        in1=el_t[:, c0 + g * KS : c0 + g * KS + ncg].to_broadcast(
                            [128, ncg, WIN]
                        ),
                        op=mybir.AluOpType.is_equal,
                    )
                    nc.vector.tensor_tensor(
                        out=smv,
                        in0=smv,
                        in1=ev_t[:, c0 + g * KS : c0 + g * KS + ncg].to_broadcast(
                            [128, ncg, WIN]
                        ),
                        op=mybir.AluOpType.mult,
                    )
                # ScalarE builds the tail columns: Square(iota-lrow) ->
                # Relu(1-sq) -> *val  (exact for integer iota/lrow)
                for lc in range(dve_cols, ncols):
                    sq = actp.tile([128, WIN], dt_edge, tag="sq")
                    nc.scalar.activation(
                        sq[:],
                        iota_t[:, :WIN],
                        mybir.ActivationFunctionType.Square,
                        bias=elf_t[:, c0 + lc : c0 + lc + 1],
                        scale=-1.0,
                    )
                    oh = actp.tile([128, WIN], dt_edge, tag="oh")
                    nc.scalar.activation(
                        oh[:],
                        sq[:],
                        mybir.ActivationFunctionType.Relu,
                        bias=1.0,
                        scale=-1.0,
                    )
                    nc.scalar.mul(
                        sm[:, lc * WIN : (lc + 1) * WIN],
                        oh[:],
                        evf_t[:, c0 + lc : c0 + lc + 1],
                    )

                for w in range(w0, w0 + nwin):
                    wcols = win_cols[w]
                    psum1 = psum.tile([C, WIN], f32, tag="psum1")
                    for ti, col in enumerate(wcols):
                        lc = col - c0
                        if TRS and lc < (ncols - act_cols):
                            g, cc = lc // KS, lc % KS
                            rhs_ap = sm[
                                :, g * KS * WIN : (g + 1) * KS * WIN
                            ].rearrange("p (i c) -> p i c", i=WIN, c=KS)[:, :, cc]
                        else:
                            rhs_ap = sm[:, lc * WIN : (lc + 1) * WIN]
                        nc.tensor.matmul(
                            psum1[:],
                            lhsT=xg[:, lc, :],
                            rhs=rhs_ap,
                            start=(ti == 0),
                            stop=(ti == len(wcols) - 1),
                        )

                    # psum1 holds aggT [cin, rows]; out_w = aggT.T @ W.T + b
                    # (the bias lands via a rank-1 matmul; PSUM->SBUF copies
                    # run on the Scalar engine so DVE stays free for S builds)
                    aggT = op.tile([C, WIN], f32, tag="aggT")
                    nc.scalar.copy(aggT[:], psum1[:])
                    psum2 = psum.tile([WIN, C], f32, tag="psum2")
                    nc.tensor.matmul(
                        psum2[:], lhsT=aggT[:], rhs=wt_t[:], start=True, stop=False
                    )
                    nc.tensor.matmul(
                        psum2[:], lhsT=ones_t[:], rhs=bias_t[:], start=False, stop=True
                    )
                    outw = op.tile([WIN, C], f32, tag="outw")
                    nc.scalar.copy(outw[:], psum2[:])
                    nc.sync.dma_start(out_d[w * WIN : (w + 1) * WIN, :], outw[:])

    nc.compile()
    return nc


def kernel(x, rows, cols, vals, W, b):
    from concourse.bass_utils import run_bass_kernel_spmd

    NW, _ = _derived()
    x = np.ascontiguousarray(np.asarray(x), dtype=np.float32)
    W = np.asarray(W).astype(np.float32)
    b = np.asarray(b).astype(np.float32)

    eidx16, ev, el, batches, calls, win_cols, tc_total = _host_prep(rows, cols, vals)

    if EDGE_DTYPE == "bf16":
        import ml_dtypes

        x_dev = x.astype(ml_dtypes.bfloat16)
        mdt = ml_dtypes.bfloat16
    else:
        x_dev = x
        mdt = np.float32
    iota = np.ascontiguousarray(
        np.broadcast_to(
            np.tile(-np.arange(WIN, dtype=np.float32), KS), (WIN, KS * WIN)
        )
    ).astype(mdt)

    iota_rep = np.ascontiguousarray(
        np.broadcast_to(
            np.repeat(-np.arange(WIN, dtype=np.float32), KS), (128, WIN * KS)
        )
    ).astype(mdt)
    wt = np.ascontiguousarray(W.T)  # [cin, cout]
    bias_rep = np.ascontiguousarray(b.reshape(1, C))
    ones_row = np.ones((1, WIN), dtype=np.float32)

    nc = _build_program(batches, calls, win_cols, tc_total, EDGE_DTYPE)

    in_maps = [
        {
            "xin": x_dev,
            "eidx": np.ascontiguousarray(eidx16[c]),
            "ev": ev[c].astype(mdt),
            "el": el[c].astype(mdt),
            **({"evf": ev[c], "elf": el[c]} if (ACT_FRAC > 0 or TSCOL) else {}),
            "wt": wt,
            "bias": bias_rep,
            "ones": ones_row,
            "iota": np.ascontiguousarray(iota),
            "iotar": iota_rep,
        }
        for c in range(N_CORES)
    ]

    res = run_bass_kernel_spmd(nc, in_maps, list(range(N_CORES)), trace=TRACE)
    LAST_RESULT["exec_time_ns"] = res.exec_time_ns
    LAST_RESULT["results"] = res

    out = np.empty((N_NODES, C), dtype=np.float32)
    for c in range(N_CORES):
        out[c * RPC : (c + 1) * RPC] = res.results[c]["out"][:RPC]
    return out
